# revision 5
# baseline (speedup 1.0000x reference)
"""DiffGCN on 8 Trainium2 NeuronCores (Bass/Tile).

Sharding: nodes/dst-ranges across 8 cores (12544 nodes each, padded to
100352 = 784*128). Edges are sharded by dst range and binned by dst block
(128 nodes) on the host; src features are halo-exchanged per edge (host
gather of x[src], deg[src], u[src], v[dst] — the data plane of the
distributed GNN). All FLOPs run on device:

L1: deg histogram per dst shard  (one-hot fp8 + PE matmul accumulate)
L2: per-edge g = relu(x@We+be)@Wg * rsqrt(deg+1)  (PE/ACT), scatter-add
    segment sum via one-hot matmul into PSUM, then h/u/v per node.
L3: scores = sigmoid(u[src] + v[dst] + b)  (DVE/ACT elementwise)
"""
import numpy as np

import concourse.bass as bass
import concourse.mybir as mybir
import concourse.tile as tile
from concourse.bass_utils import run_bass_kernel_spmd
from concourse.tile import ScopedClock

DT = mybir.dt
P = 128
NC = 8
N = 100000
E = 3200000
NBLK = 98                # dst blocks per core
NPC = NBLK * P           # 12544 nodes per core
NPAD = NC * NPC          # 100352
NT = NPAD // P           # 784 node tiles
CPB = 36                 # chunks of 128 edges per dst block (4608 slots)
BPAD = CPB * P
E2 = NBLK * BPAD         # 451584 edge slots per core (L2)
NCH = E2 // P            # 3528 chunks
KB = 8                   # chunks per one-hot batch
NJ = NCH // KB           # 441 batches
CW = 512                 # columns per edge-encoder matmul group
E3 = E // NC             # 400000 (exact) edges per core (L3)
NJ3 = E3 // P            # 3125

LAST_EXEC_NS = []

# ---------------------------------------------------------------------------
# walrus in this container encodes at most ONE sync-wait per instruction;
# split multi-wait instructions into single-wait NOPs. Also keep the Tile
# tail drain single-wait.
_split_n = [0]


def _split_multi_waits(nc):
    for f in nc.m.functions:
        for bb in f.blocks:
            insts = bb.instructions
            out = []
            changed = False
            for inst in insts:
                si = getattr(inst, "sync_info", None)
                if si is not None and si.on_wait is not None and len(si.on_wait) > 1:
                    waits = list(si.on_wait)
                    for w in waits[:-1]:
                        _split_n[0] += 1
                        nop = mybir.InstNoOp(
                            name=f"I-wsplit-{_split_n[0]}",
                            engine=inst.engine,
                            ins=[], outs=[],
                            sync_info=mybir.SyncInfo(on_wait=[w], on_update=[]),
                        )
                        nc.register_instruction(nop, overwrite=True)
                        out.append(nop)
                    si.on_wait.clear()
                    si.on_wait.append(waits[-1])
                    changed = True
                out.append(inst)
            if changed:
                insts[:] = out


def _patched_drain_and_barrier(self, tick_clock, wait_clock):
    probe = self.nc.sync.nop(hint="drain_waits", nofuse=True)
    wait_clock.add_sem_waits(probe.ins, ScopedClock({None: tick_clock.global_clock}))
    si = probe.ins.sync_info
    waits = list(si.on_wait) if si is not None else []
    if si is not None and len(waits) > 1:
        si.on_wait.clear()
        si.on_wait.append(waits[0])
        for w in waits[1:]:
            extra = self.nc.sync.nop(hint="drain_waits", nofuse=True)
            esi = extra.ins.sync_info
            if esi is None:
                extra.ins.sync_info = mybir.SyncInfo(on_wait=[w], on_update=[])
            else:
                esi.on_wait.append(w)
    self.nc.sync.drain()
    self.nc.all_engine_barrier()
    assert self.sems is not None
    popped = self.nc._tile_sem_poison_stack.pop()
    assert popped is self._sem_poison
    self.nc.clear_and_free_semaphores(list(self.sems.allocated().values()))
    self.nc.all_engine_barrier()


tile.TileContext._drain_and_barrier = _patched_drain_and_barrier


# ---------------------------------------------------------------------------
def _build_l1():
    """deg histogram: dstlo [128, NCH] bf16 -> deg [128, NBLK] f32."""
    nc = bass.Bass("TRN2", debug=False, num_devices=NC)
    dstlo = nc.dram_tensor("dstlo", [P, NCH], DT.bfloat16, kind="ExternalInput")
    iota_in = nc.dram_tensor("iota_in", [P, P], DT.bfloat16, kind="ExternalInput")
    deg_out = nc.dram_tensor("deg_out", [P, NBLK], DT.float32, kind="ExternalOutput")
    with tile.TileContext(nc) as tc:
        with (
            tc.tile_pool(name="sbuf", bufs=3) as pool,
            tc.tile_pool(name="big", bufs=1) as big,
            tc.tile_pool(name="ps", bufs=1, space="PSUM") as ps,
        ):
            iota_t = big.tile([P, P], DT.bfloat16)
            nc.sync.dma_start(out=iota_t[:], in_=iota_in[:])
            lo_all = big.tile([P, NCH], DT.bfloat16)
            nc.sync.dma_start(out=lo_all[:], in_=dstlo[:])
            ones_t = big.tile([P, 1], DT.float8e4)
            nc.gpsimd.memset(ones_t[:], 1.0)
            deg_psum = ps.tile([P, NBLK], DT.float32)
            for j in range(NJ):
                oh8 = pool.tile([P, KB, P], DT.float8e4, tag="oh8")
                nc.vector.tensor_tensor(
                    out=oh8[:],
                    in0=lo_all[:, j * KB:(j + 1) * KB, None].to_broadcast([P, KB, P]),
                    in1=iota_t[:].rearrange("p (o c) -> p o c", o=1)
                        .to_broadcast([P, KB, P]),
                    op=mybir.AluOpType.is_equal,
                )
                for k in range(KB):
                    ch = j * KB + k
                    b, r = ch // CPB, ch % CPB
                    nc.tensor.matmul(
                        out=deg_psum[:, b:b + 1], lhsT=oh8[:, k, :], rhs=ones_t[:],
                        start=(r == 0), stop=(r == CPB - 1),
                    )
            deg_sb = big.tile([P, NBLK], DT.float32)
            nc.vector.tensor_copy(out=deg_sb[:], in_=deg_psum[:])
            nc.sync.dma_start(out=deg_out[:], in_=deg_sb[:])
    _split_multi_waits(nc)
    return nc


def _build_l2():
    """Edge aggregation + node update.

    inputs:
      exT    [7, E2]      f32  edge-halo'd x[src] (chunk-major columns)
      edeg   [P, NCH]     f32  edge-halo'd deg[src]
      dstlo  [P, NCH]     bf16 local-dst low 7 bits (200 = pad)
      xcT    [7, NPC]     f32  local nodes' x
      degc   [P, NBLK]    f32  local deg (from L1)
      wenc   [7, 32], benc [32,1] bcast, wgcn [32, 32], bgcn_r [P, 32],
      wu_r   [P, 32], wv_r [P, 32]  (bias/W_edge replicated per partition)
      iota_in [P, P] bf16, id32 [32, 32] f32
    outputs: u_out, v_out [P, NBLK] f32
    """
    nc = bass.Bass("TRN2", debug=False, num_devices=NC)
    exT = nc.dram_tensor("exT", [7, E2], DT.float32, kind="ExternalInput")
    edeg = nc.dram_tensor("edeg", [P, NCH], DT.float32, kind="ExternalInput")
    dstlo = nc.dram_tensor("dstlo", [P, NCH], DT.bfloat16, kind="ExternalInput")
    xcT = nc.dram_tensor("xcT", [7, NPC], DT.float32, kind="ExternalInput")
    degc = nc.dram_tensor("degc", [P, NBLK], DT.float32, kind="ExternalInput")
    wenc = nc.dram_tensor("wenc", [7, 32], DT.float32, kind="ExternalInput")
    benc = nc.dram_tensor("benc", [32, 1], DT.float32, kind="ExternalInput")
    wgcn = nc.dram_tensor("wgcn", [32, 32], DT.float32, kind="ExternalInput")
    bgcn_r = nc.dram_tensor("bgcn_r", [P, 32], DT.float32, kind="ExternalInput")
    wu_r = nc.dram_tensor("wu_r", [P, 32], DT.float32, kind="ExternalInput")
    wv_r = nc.dram_tensor("wv_r", [P, 32], DT.float32, kind="ExternalInput")
    iota_in = nc.dram_tensor("iota_in", [P, P], DT.bfloat16, kind="ExternalInput")
    id32 = nc.dram_tensor("id32", [32, 32], DT.float32, kind="ExternalInput")
    u_out = nc.dram_tensor("u_out", [P, NBLK], DT.float32, kind="ExternalOutput")
    v_out = nc.dram_tensor("v_out", [P, NBLK], DT.float32, kind="ExternalOutput")

    GPB = CW // P            # 4 chunks per encoder group
    NG = E2 // CW            # 882 encoder groups
    HALF = NBLK // 2         # 49 blocks per psum half

    with tile.TileContext(nc) as tc:
        with (
            tc.tile_pool(name="cons", bufs=1) as cons,
            tc.tile_pool(name="pool", bufs=2) as pool,
            tc.tile_pool(name="pex", bufs=2) as pex,
            tc.tile_pool(name="ps1", bufs=1, space="PSUM") as ps1,
            tc.tile_pool(name="ps3", bufs=1, space="PSUM") as ps3,
            tc.tile_pool(name="pss", bufs=1, space="PSUM") as pss,
        ):
            # constants
            iota_t = cons.tile([P, P], DT.bfloat16)
            nc.sync.dma_start(out=iota_t[:], in_=iota_in[:])
            we_t = cons.tile([7, 32], DT.float32)
            nc.sync.dma_start(out=we_t[:], in_=wenc[:])
            be_t = cons.tile([32, 1], DT.float32)
            nc.sync.dma_start(out=be_t[:], in_=benc[:])
            wg_t = cons.tile([32, 32], DT.float32)
            nc.sync.dma_start(out=wg_t[:], in_=wgcn[:])
            id_t = cons.tile([32, 32], DT.float32)
            nc.sync.dma_start(out=id_t[:], in_=id32[:])
            bg_t = cons.tile([P, 32], DT.float32)
            nc.sync.dma_start(out=bg_t[:], in_=bgcn_r[:])
            wu_t = cons.tile([P, 32], DT.float32)
            nc.sync.dma_start(out=wu_t[:], in_=wu_r[:])
            wv_t = cons.tile([P, 32], DT.float32)
            nc.sync.dma_start(out=wv_t[:], in_=wv_r[:])
            lo_all = cons.tile([P, NCH], DT.bfloat16)
            nc.sync.dma_start(out=lo_all[:], in_=dstlo[:])

            # edge dinv = rsqrt(edeg + 1)
            edinv = cons.tile([P, NCH], DT.float32)
            nc.sync.dma_start(out=edinv[:], in_=edeg[:])
            nc.scalar.activation(out=edinv[:], in_=edinv[:],
                                 func=mybir.ActivationFunctionType.Sqrt, bias=1.0)
            nc.vector.reciprocal(out=edinv[:], in_=edinv[:])

            # local dinv = rsqrt(degc + 1)
            dinvc = cons.tile([P, NBLK], DT.float32)
            nc.sync.dma_start(out=dinvc[:], in_=degc[:])
            nc.scalar.activation(out=dinvc[:], in_=dinvc[:],
                                 func=mybir.ActivationFunctionType.Sqrt, bias=1.0)
            nc.vector.reciprocal(out=dinvc[:], in_=dinvc[:])

            s_sb = cons.tile([P, NBLK * 32], DT.float32)

            # ---- edge sweep: encoder + transpose + scale + one-hot + scatter
            SLABG = 7                # groups per slab
            SLAB = SLABG * CW        # 10752 cols
            for half in range(2):
                s_psum = pss.tile([P, HALF * 32], DT.float32, tag="s")
                for g in range(NG // 2):
                    g_abs = half * (NG // 2) + g
                    c0 = g_abs * CW
                    if g % SLABG == 0:
                        ex_sb = pex.tile([7, SLAB], DT.float32, tag="exsb")
                        nc.sync.dma_start(out=ex_sb[:],
                                          in_=exT[:, c0:c0 + SLAB])
                    cs = (g % SLABG) * CW
                    h1p = ps1.tile([32, CW], DT.float32, tag="h1")
                    nc.tensor.matmul(out=h1p[:], lhsT=we_t[:],
                                     rhs=ex_sb[:, cs:cs + CW],
                                     start=True, stop=True)
                    h1s = pool.tile([32, CW], DT.float32, tag="h1s")
                    nc.scalar.activation(out=h1s[:], in_=h1p[:],
                                         func=mybir.ActivationFunctionType.Relu,
                                         bias=be_t[:])
                    h2p = ps1.tile([32, CW], DT.float32, tag="h2")
                    nc.tensor.matmul(out=h2p[:], lhsT=wg_t[:], rhs=h1s[:],
                                     start=True, stop=True)
                    h2s = pool.tile([32, CW], DT.float32, tag="h2s")
                    nc.vector.tensor_copy(out=h2s[:], in_=h2p[:])
                    # transpose 4 chunks into [128, 4, 32] psum
                    tp = ps3.tile([P, GPB, 32], DT.float32, tag="tp")
                    for t in range(GPB):
                        nc.tensor.transpose(
                            out=tp[:, t, :], in_=h2s[:, t * P:(t + 1) * P],
                            identity=id_t[:])
                    # scale by edinv, cast bf16
                    ch_s = g_abs * GPB
                    grhs = pool.tile([P, GPB, 32], DT.bfloat16, tag="grhs")
                    nc.vector.tensor_tensor(
                        out=grhs[:], in0=tp[:],
                        in1=edinv[:, ch_s:ch_s + GPB, None].to_broadcast([P, GPB, 32]),
                        op=mybir.AluOpType.mult)
                    # one-hot for these 4 chunks
                    oh = pool.tile([P, GPB, P], DT.bfloat16, tag="oh")
                    nc.vector.tensor_tensor(
                        out=oh[:],
                        in0=lo_all[:, ch_s:ch_s + GPB, None].to_broadcast([P, GPB, P]),
                        in1=iota_t[:].rearrange("p (o c) -> p o c", o=1)
                            .to_broadcast([P, GPB, P]),
                        op=mybir.AluOpType.is_equal)
                    for t in range(GPB):
                        ch = ch_s + t
                        b, r = ch // CPB, ch % CPB
                        nc.tensor.matmul(
                            out=s_psum[:, (b - half * HALF) * 32:(b - half * HALF + 1) * 32],
                            lhsT=oh[:, t, :], rhs=grhs[:, t, :],
                            start=(r == 0), stop=(r == CPB - 1))
                nc.vector.tensor_copy(out=s_sb[:, half * HALF * 32:(half + 1) * HALF * 32],
                                      in_=s_psum[:])

            # ---- local nodes: h2_local via same chain
            xc_sb = cons.tile([7, NPC], DT.float32)
            nc.sync.dma_start(out=xc_sb[:], in_=xcT[:])
            g_loc = cons.tile([P, NBLK, 32], DT.float32)
            NGL = NPC // CW      # 24.5 -> use 128-col groups for locals
            NGL = NPC // P       # 98 tiles of 128
            for g in range(NGL // GPB):
                c0 = g * CW
                h1p = ps1.tile([32, CW], DT.float32, tag="h1")
                nc.tensor.matmul(out=h1p[:], lhsT=we_t[:], rhs=xc_sb[:, c0:c0 + CW],
                                 start=True, stop=True)
                h1s = pool.tile([32, CW], DT.float32, tag="h1s")
                nc.scalar.activation(out=h1s[:], in_=h1p[:],
                                     func=mybir.ActivationFunctionType.Relu,
                                     bias=be_t[:])
                h2p = ps1.tile([32, CW], DT.float32, tag="h2")
                nc.tensor.matmul(out=h2p[:], lhsT=wg_t[:], rhs=h1s[:],
                                 start=True, stop=True)
                h2s = pool.tile([32, CW], DT.float32, tag="h2s")
                nc.vector.tensor_copy(out=h2s[:], in_=h2p[:])
                tp = ps3.tile([P, GPB, 32], DT.float32, tag="tp")
                for t in range(GPB):
                    nc.tensor.transpose(out=tp[:, t, :], in_=h2s[:, t * P:(t + 1) * P],
                                        identity=id_t[:])
                blk0 = g * GPB
                nc.vector.tensor_tensor(
                    out=g_loc[:, blk0:blk0 + GPB, :], in0=tp[:],
                    in1=dinvc[:, blk0:blk0 + GPB, None].to_broadcast([P, GPB, 32]),
                    op=mybir.AluOpType.mult)
            # remaining 98 - 96 = 2 tiles
            rem = NGL - (NGL // GPB) * GPB
            if rem:
                c0 = (NGL // GPB) * CW
                h1p = ps1.tile([32, rem * P], DT.float32, tag="h1")
                nc.tensor.matmul(out=h1p[:], lhsT=we_t[:], rhs=xc_sb[:, c0:c0 + rem * P],
                                 start=True, stop=True)
                h1s = pool.tile([32, rem * P], DT.float32, tag="h1s2")
                nc.scalar.activation(out=h1s[:], in_=h1p[:],
                                     func=mybir.ActivationFunctionType.Relu,
                                     bias=be_t[:])
                h2p = ps1.tile([32, rem * P], DT.float32, tag="h2")
                nc.tensor.matmul(out=h2p[:], lhsT=wg_t[:], rhs=h1s[:],
                                 start=True, stop=True)
                h2s = pool.tile([32, rem * P], DT.float32, tag="h2s2")
                nc.vector.tensor_copy(out=h2s[:], in_=h2p[:])
                tp = ps3.tile([P, rem, 32], DT.float32, tag="tp")
                for t in range(rem):
                    nc.tensor.transpose(out=tp[:, t, :], in_=h2s[:, t * P:(t + 1) * P],
                                        identity=id_t[:])
                blk0 = (NGL // GPB) * GPB
                nc.vector.tensor_tensor(
                    out=g_loc[:, blk0:blk0 + rem, :], in0=tp[:],
                    in1=dinvc[:, blk0:blk0 + rem, None].to_broadcast([P, rem, 32]),
                    op=mybir.AluOpType.mult)

            # ---- h = relu(dinv * (s + g_loc) + bgcn); u, v   (in place on s_sb)
            hsum = s_sb[:].rearrange("p (b f) -> p b f", f=32)
            nc.vector.tensor_tensor(out=hsum, in0=hsum, in1=g_loc[:],
                                    op=mybir.AluOpType.add)
            nc.vector.tensor_tensor(
                out=hsum, in0=hsum,
                in1=dinvc[:, :, None].to_broadcast([P, NBLK, 32]),
                op=mybir.AluOpType.mult)
            nc.vector.tensor_tensor(
                out=hsum, in0=hsum,
                in1=bg_t[:].rearrange("p (o f) -> p o f", o=1)
                    .to_broadcast([P, NBLK, 32]),
                op=mybir.AluOpType.add)
            h_t = cons.tile([P, NBLK, 32], DT.float32)
            nc.scalar.activation(out=h_t[:], in_=hsum,
                                 func=mybir.ActivationFunctionType.Relu)
            # u = sum_f h*wu ; v = sum_f h*wv
            for (w_t, o_t) in ((wu_t, u_out), (wv_t, v_out)):
                tmp = pool.tile([P, NBLK, 32], DT.float32, tag="uvtmp")
                nc.vector.tensor_tensor(
                    out=tmp[:], in0=h_t[:],
                    in1=w_t[:].rearrange("p (o f) -> p o f", o=1)
                        .to_broadcast([P, NBLK, 32]),
                    op=mybir.AluOpType.mult)
                red = pool.tile([P, NBLK], DT.float32, tag="uvred")
                nc.vector.tensor_reduce(out=red[:], in_=tmp[:],
                                        axis=mybir.AxisListType.X,
                                        op=mybir.AluOpType.add)
                nc.sync.dma_start(out=o_t[:], in_=red[:])
    _split_multi_waits(nc)
    return nc


def _build_l3():
    """scores = sigmoid(eu + ev + b_edge)."""
    nc = bass.Bass("TRN2", debug=False, num_devices=NC)
    eu = nc.dram_tensor("eu", [P, NJ3], DT.float32, kind="ExternalInput")
    ev = nc.dram_tensor("ev", [P, NJ3], DT.float32, kind="ExternalInput")
    bedge = nc.dram_tensor("bedge", [P, 1], DT.float32, kind="ExternalInput")
    sc = nc.dram_tensor("sc", [P, NJ3], DT.float32, kind="ExternalOutput")
    with tile.TileContext(nc) as tc:
        with tc.tile_pool(name="pool", bufs=1) as pool:
            eu_t = pool.tile([P, NJ3], DT.float32)
            nc.sync.dma_start(out=eu_t[:], in_=eu[:])
            ev_t = pool.tile([P, NJ3], DT.float32)
            nc.sync.dma_start(out=ev_t[:], in_=ev[:])
            b_t = pool.tile([P, 1], DT.float32)
            nc.sync.dma_start(out=b_t[:], in_=bedge[:])
            su = pool.tile([P, NJ3], DT.float32)
            nc.vector.tensor_tensor(out=su[:], in0=eu_t[:], in1=ev_t[:],
                                    op=mybir.AluOpType.add)
            sg = pool.tile([P, NJ3], DT.float32)
            nc.scalar.activation(out=sg[:], in_=su[:],
                                 func=mybir.ActivationFunctionType.Sigmoid,
                                 bias=b_t[:])
            nc.sync.dma_start(out=sc[:], in_=sg[:])
    _split_multi_waits(nc)
    return nc


_CACHE = {}


def _get(name, builder):
    if name not in _CACHE:
        _CACHE[name] = builder()
    return _CACHE[name]


def kernel(x_t, x_t_dt, edge_index, W_enc, b_enc, W_gcn, b_gcn, W_edge, b_edge):
    import ml_dtypes
    bf16 = ml_dtypes.bfloat16
    x_t = np.asarray(x_t, dtype=np.float32)
    W_enc = np.asarray(W_enc, np.float32)
    b_enc = np.asarray(b_enc, np.float32)
    W_gcn = np.asarray(W_gcn, np.float32)
    b_gcn = np.asarray(b_gcn, np.float32)
    W_edge = np.asarray(W_edge, np.float32)
    b_edge = np.asarray(b_edge, np.float32)
    src = np.asarray(edge_index[0], np.int64).astype(np.int32)
    dst = np.asarray(edge_index[1], np.int64).astype(np.int32)
    del LAST_EXEC_NS[:]

    iota = np.tile(np.arange(P, dtype=np.float32).astype(bf16).reshape(1, P), (P, 1))

    # ---- shard edges by dst range, bin by dst block (host-side sharding) ----
    core = dst // NPC
    blk_g = dst // P                    # global block id (core*98 + local blk)
    order = np.argsort(blk_g, kind="stable")
    src_o, dst_o = src[order], dst[order]
    blk_o = blk_g[order]
    counts = np.bincount(blk_o, minlength=NC * NBLK)
    assert counts.max() <= BPAD, f"block overflow {counts.max()} > {BPAD}"
    # slot each edge into its block's padded region
    starts = np.zeros(NC * NBLK, np.int64)
    starts[1:] = np.cumsum(counts)[:-1]
    within = np.arange(E) - starts[blk_o]
    slot_g = blk_o * BPAD + within       # global padded slot (core-major)

    # per-core padded edge arrays
    e_src = np.zeros((NC, E2), np.int32)
    e_lo = np.full((NC, E2), 200.0, np.float32)
    c_o = blk_o // NBLK
    slot_l = slot_g - c_o * E2
    e_src[c_o, slot_l] = src_o
    e_lo[c_o, slot_l] = (dst_o % P).astype(np.float32)

    # chunk-major [p, ch] layouts
    def pch(a):      # [NC, E2] -> [NC, P, NCH] with [c, p, ch] = a[c, ch*128+p]
        return np.ascontiguousarray(a.reshape(NC, NCH, P).transpose(0, 2, 1))

    e_lo_pch = pch(e_lo).astype(bf16)

    # ---- L1: degree histogram ----
    nc1 = _get("l1", _build_l1)
    in_maps = [{"dstlo": e_lo_pch[c], "iota_in": iota} for c in range(NC)]
    res1 = run_bass_kernel_spmd(nc1, in_maps, core_ids=list(range(NC)))
    if res1.exec_time_ns:
        LAST_EXEC_NS.append(res1.exec_time_ns)
    deg_full = np.zeros(NPAD, np.float32)
    for c in range(NC):
        d = res1.results[c]["deg_out"]      # [p, blk]
        deg_full[c * NPC:(c + 1) * NPC] = d.T.reshape(-1)

    # ---- L2 prep: halo-exchange per-edge src features ----
    xpad = np.zeros((NPAD, 7), np.float32)
    xpad[:N] = x_t
    ex = xpad[e_src.reshape(-1)].reshape(NC, E2, 7)
    exT = np.ascontiguousarray(ex.transpose(0, 2, 1))          # [NC, 7, E2]
    edeg = pch(deg_full[e_src.reshape(-1)].reshape(NC, E2).astype(np.float32))
    xcT = np.ascontiguousarray(
        xpad.reshape(NC, NPC, 7).transpose(0, 2, 1))           # [NC, 7, NPC]
    degc = np.ascontiguousarray(
        deg_full.reshape(NC, NBLK, P).transpose(0, 2, 1))      # [NC, p, blk]

    wu = W_edge[:32, 0].astype(np.float32)
    wv = W_edge[32:, 0].astype(np.float32)
    common = {
        "wenc": W_enc, "benc": b_enc.reshape(32, 1),
        "wgcn": W_gcn, "bgcn_r": np.tile(b_gcn.reshape(1, 32), (P, 1)),
        "wu_r": np.tile(wu.reshape(1, 32), (P, 1)),
        "wv_r": np.tile(wv.reshape(1, 32), (P, 1)),
        "iota_in": iota, "id32": np.eye(32, dtype=np.float32),
    }
    nc2 = _get("l2", _build_l2)
    in_maps = [dict(common, exT=exT[c], edeg=edeg[c], dstlo=e_lo_pch[c],
                    xcT=xcT[c], degc=degc[c]) for c in range(NC)]
    res2 = run_bass_kernel_spmd(nc2, in_maps, core_ids=list(range(NC)))
    if res2.exec_time_ns:
        LAST_EXEC_NS.append(res2.exec_time_ns)
    u_full = np.zeros(NPAD, np.float32)
    v_full = np.zeros(NPAD, np.float32)
    for c in range(NC):
        u_full[c * NPC:(c + 1) * NPC] = res2.results[c]["u_out"].T.reshape(-1)
        v_full[c * NPC:(c + 1) * NPC] = res2.results[c]["v_out"].T.reshape(-1)

    # ---- L3: edge scorer ----
    # original edge order; core c takes edges [c*E3, (c+1)*E3)
    eu = u_full[src].reshape(NC, NJ3, P).transpose(0, 2, 1)
    ev = v_full[dst].reshape(NC, NJ3, P).transpose(0, 2, 1)
    eu = np.ascontiguousarray(eu)
    ev = np.ascontiguousarray(ev)
    nc3 = _get("l3", _build_l3)
    bvec = np.full((P, 1), float(b_edge.reshape(-1)[0]), np.float32)
    in_maps = [{"eu": eu[c], "ev": ev[c], "bedge": bvec} for c in range(NC)]
    res3 = run_bass_kernel_spmd(nc3, in_maps, core_ids=list(range(NC)))
    if res3.exec_time_ns:
        LAST_EXEC_NS.append(res3.exec_time_ns)
    scores = np.zeros(E, np.float32)
    for c in range(NC):
        sc = res3.results[c]["sc"]          # [p, j]
        scores[c * E3:(c + 1) * E3] = sc.T.reshape(-1)
    return scores


# revision 6
# speedup vs baseline: 1.1076x; 1.1076x over previous
"""DiffGCN on 8 Trainium2 NeuronCores (Bass/Tile).

Sharding: nodes/dst-ranges across 8 cores (12544 nodes each, padded to
100352 = 784*128). Edges are sharded by dst range and binned by dst block
(128 nodes) on the host; src features are halo-exchanged per edge (host
gather of x[src], deg[src], u[src], v[dst] — the data plane of the
distributed GNN). All FLOPs run on device:

L1: deg histogram per dst shard  (one-hot fp8 + PE matmul accumulate)
L2: per-edge g = relu(x@We+be)@Wg * rsqrt(deg+1)  (PE/ACT), scatter-add
    segment sum via one-hot matmul into PSUM, then h/u/v per node.
L3: scores = sigmoid(u[src] + v[dst] + b)  (DVE/ACT elementwise)
"""
import numpy as np

import concourse.bass as bass
import concourse.mybir as mybir
import concourse.tile as tile
from concourse.bass_utils import run_bass_kernel_spmd
from concourse.tile import ScopedClock

DT = mybir.dt
P = 128
NC = 8
N = 100000
E = 3200000
NBLK = 98                # dst blocks per core
NPC = NBLK * P           # 12544 nodes per core
NPAD = NC * NPC          # 100352
NT = NPAD // P           # 784 node tiles
CPB = 36                 # chunks of 128 edges per dst block (4608 slots)
BPAD = CPB * P
E2 = NBLK * BPAD         # 451584 edge slots per core (L2)
NCH = E2 // P            # 3528 chunks
KB = 8                   # chunks per one-hot batch
NJ = NCH // KB           # 441 batches
CW = 512                 # columns per edge-encoder matmul group
E3 = E // NC             # 400000 (exact) edges per core (L3)
NJ3 = E3 // P            # 3125

LAST_EXEC_NS = []

# ---------------------------------------------------------------------------
# walrus in this container encodes at most ONE sync-wait per instruction;
# split multi-wait instructions into single-wait NOPs. Also keep the Tile
# tail drain single-wait.
_split_n = [0]


def _split_multi_waits(nc):
    for f in nc.m.functions:
        for bb in f.blocks:
            insts = bb.instructions
            out = []
            changed = False
            for inst in insts:
                si = getattr(inst, "sync_info", None)
                if si is not None and si.on_wait is not None and len(si.on_wait) > 1:
                    waits = list(si.on_wait)
                    for w in waits[:-1]:
                        _split_n[0] += 1
                        nop = mybir.InstNoOp(
                            name=f"I-wsplit-{_split_n[0]}",
                            engine=inst.engine,
                            ins=[], outs=[],
                            sync_info=mybir.SyncInfo(on_wait=[w], on_update=[]),
                        )
                        nc.register_instruction(nop, overwrite=True)
                        out.append(nop)
                    si.on_wait.clear()
                    si.on_wait.append(waits[-1])
                    changed = True
                out.append(inst)
            if changed:
                insts[:] = out


def _patched_drain_and_barrier(self, tick_clock, wait_clock):
    probe = self.nc.sync.nop(hint="drain_waits", nofuse=True)
    wait_clock.add_sem_waits(probe.ins, ScopedClock({None: tick_clock.global_clock}))
    si = probe.ins.sync_info
    waits = list(si.on_wait) if si is not None else []
    if si is not None and len(waits) > 1:
        si.on_wait.clear()
        si.on_wait.append(waits[0])
        for w in waits[1:]:
            extra = self.nc.sync.nop(hint="drain_waits", nofuse=True)
            esi = extra.ins.sync_info
            if esi is None:
                extra.ins.sync_info = mybir.SyncInfo(on_wait=[w], on_update=[])
            else:
                esi.on_wait.append(w)
    self.nc.sync.drain()
    self.nc.all_engine_barrier()
    assert self.sems is not None
    popped = self.nc._tile_sem_poison_stack.pop()
    assert popped is self._sem_poison
    self.nc.clear_and_free_semaphores(list(self.sems.allocated().values()))
    self.nc.all_engine_barrier()


tile.TileContext._drain_and_barrier = _patched_drain_and_barrier


# ---------------------------------------------------------------------------
def _build_l1():
    """deg histogram: dstlo [128, NCH] bf16 -> deg [128, NBLK] f32."""
    nc = bass.Bass("TRN2", debug=False, num_devices=NC)
    dstlo = nc.dram_tensor("dstlo", [P, NCH], DT.bfloat16, kind="ExternalInput")
    iota_in = nc.dram_tensor("iota_in", [P, P], DT.bfloat16, kind="ExternalInput")
    deg_out = nc.dram_tensor("deg_out", [P, NBLK], DT.float32, kind="ExternalOutput")
    with tile.TileContext(nc) as tc:
        with (
            tc.tile_pool(name="sbuf", bufs=3) as pool,
            tc.tile_pool(name="big", bufs=1) as big,
            tc.tile_pool(name="ps", bufs=1, space="PSUM") as ps,
        ):
            iota_t = big.tile([P, P], DT.bfloat16)
            nc.sync.dma_start(out=iota_t[:], in_=iota_in[:])
            lo_all = big.tile([P, NCH], DT.bfloat16)
            nc.sync.dma_start(out=lo_all[:], in_=dstlo[:])
            ones_t = big.tile([P, 1], DT.float8e4)
            nc.gpsimd.memset(ones_t[:], 1.0)
            deg_psum = ps.tile([P, NBLK], DT.float32)
            for j in range(NJ):
                oh8 = pool.tile([P, KB, P], DT.float8e4, tag="oh8")
                nc.vector.tensor_tensor(
                    out=oh8[:],
                    in0=lo_all[:, j * KB:(j + 1) * KB, None].to_broadcast([P, KB, P]),
                    in1=iota_t[:].rearrange("p (o c) -> p o c", o=1)
                        .to_broadcast([P, KB, P]),
                    op=mybir.AluOpType.is_equal,
                )
                for k in range(KB):
                    ch = j * KB + k
                    b, r = ch // CPB, ch % CPB
                    nc.tensor.matmul(
                        out=deg_psum[:, b:b + 1], lhsT=oh8[:, k, :], rhs=ones_t[:],
                        start=(r == 0), stop=(r == CPB - 1),
                    )
            deg_sb = big.tile([P, NBLK], DT.float32)
            nc.vector.tensor_copy(out=deg_sb[:], in_=deg_psum[:])
            nc.sync.dma_start(out=deg_out[:], in_=deg_sb[:])
    _split_multi_waits(nc)
    return nc


def _build_l2():
    """Edge aggregation + node update.

    inputs:
      exT    [7, E2]      f32  edge-halo'd x[src] (chunk-major columns)
      edeg   [P, NCH]     f32  edge-halo'd deg[src]
      dstlo  [P, NCH]     bf16 local-dst low 7 bits (200 = pad)
      xcT    [7, NPC]     f32  local nodes' x
      degc   [P, NBLK]    f32  local deg (from L1)
      wenc   [7, 32], benc [32,1] bcast, wgcn [32, 32], bgcn_r [P, 32],
      wu_r   [P, 32], wv_r [P, 32]  (bias/W_edge replicated per partition)
      iota_in [P, P] bf16, id32 [32, 32] f32
    outputs: u_out, v_out [P, NBLK] f32
    """
    nc = bass.Bass("TRN2", debug=False, num_devices=NC)
    exT = nc.dram_tensor("exT", [7, E2], DT.float32, kind="ExternalInput")
    edeg = nc.dram_tensor("edeg", [P, NCH], DT.float32, kind="ExternalInput")
    dstlo = nc.dram_tensor("dstlo", [P, NCH], DT.bfloat16, kind="ExternalInput")
    xcT = nc.dram_tensor("xcT", [7, NPC], DT.float32, kind="ExternalInput")
    degc = nc.dram_tensor("degc", [P, NBLK], DT.float32, kind="ExternalInput")
    wenc = nc.dram_tensor("wenc", [7, 32], DT.float32, kind="ExternalInput")
    benc = nc.dram_tensor("benc", [32, 1], DT.float32, kind="ExternalInput")
    wgcn = nc.dram_tensor("wgcn", [32, 32], DT.float32, kind="ExternalInput")
    bgcn_r = nc.dram_tensor("bgcn_r", [P, 32], DT.float32, kind="ExternalInput")
    wu_r = nc.dram_tensor("wu_r", [P, 32], DT.float32, kind="ExternalInput")
    wv_r = nc.dram_tensor("wv_r", [P, 32], DT.float32, kind="ExternalInput")
    iota_in = nc.dram_tensor("iota_in", [P, P], DT.bfloat16, kind="ExternalInput")
    id32 = nc.dram_tensor("id32", [32, 32], DT.float32, kind="ExternalInput")
    u_out = nc.dram_tensor("u_out", [P, NBLK], DT.float32, kind="ExternalOutput")
    v_out = nc.dram_tensor("v_out", [P, NBLK], DT.float32, kind="ExternalOutput")

    GPB = CW // P            # 4 chunks per encoder group
    NG = E2 // CW            # 882 encoder groups
    NSEC = 7                 # psum sections
    SECB = NBLK // NSEC      # 14 blocks per section (1 psum bank)

    with tile.TileContext(nc) as tc:
        with (
            tc.tile_pool(name="cons", bufs=1) as cons,
            tc.tile_pool(name="pool", bufs=2) as pool,
            tc.tile_pool(name="pex", bufs=2) as pex,
            tc.tile_pool(name="ps1", bufs=2, space="PSUM") as ps1,
            tc.tile_pool(name="ps3", bufs=2, space="PSUM") as ps3,
            tc.tile_pool(name="pss", bufs=1, space="PSUM") as pss,
        ):
            # constants
            iota_t = cons.tile([P, P], DT.bfloat16)
            nc.sync.dma_start(out=iota_t[:], in_=iota_in[:])
            we_t = cons.tile([7, 32], DT.float32)
            nc.sync.dma_start(out=we_t[:], in_=wenc[:])
            be_t = cons.tile([32, 1], DT.float32)
            nc.sync.dma_start(out=be_t[:], in_=benc[:])
            wg_t = cons.tile([32, 32], DT.float32)
            nc.sync.dma_start(out=wg_t[:], in_=wgcn[:])
            id_t = cons.tile([32, 32], DT.float32)
            nc.sync.dma_start(out=id_t[:], in_=id32[:])
            bg_t = cons.tile([P, 32], DT.float32)
            nc.sync.dma_start(out=bg_t[:], in_=bgcn_r[:])
            wu_t = cons.tile([P, 32], DT.float32)
            nc.sync.dma_start(out=wu_t[:], in_=wu_r[:])
            wv_t = cons.tile([P, 32], DT.float32)
            nc.sync.dma_start(out=wv_t[:], in_=wv_r[:])
            lo_all = cons.tile([P, NCH], DT.bfloat16)
            nc.sync.dma_start(out=lo_all[:], in_=dstlo[:])

            # edge dinv = rsqrt(edeg + 1)
            edinv = cons.tile([P, NCH], DT.float32)
            nc.sync.dma_start(out=edinv[:], in_=edeg[:])
            nc.scalar.activation(out=edinv[:], in_=edinv[:],
                                 func=mybir.ActivationFunctionType.Sqrt, bias=1.0)
            nc.vector.reciprocal(out=edinv[:], in_=edinv[:])

            # local dinv = rsqrt(degc + 1)
            dinvc = cons.tile([P, NBLK], DT.float32)
            nc.sync.dma_start(out=dinvc[:], in_=degc[:])
            nc.scalar.activation(out=dinvc[:], in_=dinvc[:],
                                 func=mybir.ActivationFunctionType.Sqrt, bias=1.0)
            nc.vector.reciprocal(out=dinvc[:], in_=dinvc[:])

            s_sb = cons.tile([P, NBLK * 32], DT.float32)

            # ---- edge sweep: encoder + transpose + scale + one-hot + scatter
            SLABG = 7                # groups per slab
            SLAB = SLABG * CW        # 3584 cols
            for sec in range(NSEC):
                s_psum = pss.tile([P, SECB * 32], DT.float32, tag="s")
                for g in range(NG // NSEC):
                    g_abs = sec * (NG // NSEC) + g
                    c0 = g_abs * CW
                    if g % SLABG == 0:
                        ex_sb = pex.tile([7, SLAB], DT.float32, tag="exsb")
                        nc.sync.dma_start(out=ex_sb[:],
                                          in_=exT[:, c0:c0 + SLAB])
                    cs = (g % SLABG) * CW
                    h1p = ps1.tile([32, CW], DT.float32, tag="h1")
                    nc.tensor.matmul(out=h1p[:], lhsT=we_t[:],
                                     rhs=ex_sb[:, cs:cs + CW],
                                     start=True, stop=True)
                    h1s = pool.tile([32, CW], DT.float32, tag="h1s")
                    nc.scalar.activation(out=h1s[:], in_=h1p[:],
                                         func=mybir.ActivationFunctionType.Relu,
                                         bias=be_t[:])
                    h2p = ps1.tile([32, CW], DT.float32, tag="h2")
                    nc.tensor.matmul(out=h2p[:], lhsT=wg_t[:], rhs=h1s[:],
                                     start=True, stop=True)
                    h2s = pool.tile([32, CW], DT.float32, tag="h2s")
                    nc.vector.tensor_copy(out=h2s[:], in_=h2p[:])
                    # transpose 4 chunks into [128, 4, 32] psum
                    tp = ps3.tile([P, GPB, 32], DT.float32, tag="tp")
                    for t in range(GPB):
                        nc.tensor.transpose(
                            out=tp[:, t, :], in_=h2s[:, t * P:(t + 1) * P],
                            identity=id_t[:])
                    # scale by edinv, cast bf16
                    ch_s = g_abs * GPB
                    grhs = pool.tile([P, GPB, 32], DT.bfloat16, tag="grhs")
                    nc.vector.tensor_tensor(
                        out=grhs[:], in0=tp[:],
                        in1=edinv[:, ch_s:ch_s + GPB, None].to_broadcast([P, GPB, 32]),
                        op=mybir.AluOpType.mult)
                    # one-hot for these 4 chunks
                    oh = pool.tile([P, GPB, P], DT.bfloat16, tag="oh")
                    nc.vector.tensor_tensor(
                        out=oh[:],
                        in0=lo_all[:, ch_s:ch_s + GPB, None].to_broadcast([P, GPB, P]),
                        in1=iota_t[:].rearrange("p (o c) -> p o c", o=1)
                            .to_broadcast([P, GPB, P]),
                        op=mybir.AluOpType.is_equal)
                    for t in range(GPB):
                        ch = ch_s + t
                        b, r = ch // CPB, ch % CPB
                        bl = b - sec * SECB
                        nc.tensor.matmul(
                            out=s_psum[:, bl * 32:(bl + 1) * 32],
                            lhsT=oh[:, t, :], rhs=grhs[:, t, :],
                            start=(r == 0), stop=(r == CPB - 1))
                nc.vector.tensor_copy(out=s_sb[:, sec * SECB * 32:(sec + 1) * SECB * 32],
                                      in_=s_psum[:])

            # ---- local nodes: h2_local via same chain
            xc_sb = cons.tile([7, NPC], DT.float32)
            nc.sync.dma_start(out=xc_sb[:], in_=xcT[:])
            g_loc = cons.tile([P, NBLK, 32], DT.float32)
            NGL = NPC // CW      # 24.5 -> use 128-col groups for locals
            NGL = NPC // P       # 98 tiles of 128
            for g in range(NGL // GPB):
                c0 = g * CW
                h1p = ps1.tile([32, CW], DT.float32, tag="h1")
                nc.tensor.matmul(out=h1p[:], lhsT=we_t[:], rhs=xc_sb[:, c0:c0 + CW],
                                 start=True, stop=True)
                h1s = pool.tile([32, CW], DT.float32, tag="h1s")
                nc.scalar.activation(out=h1s[:], in_=h1p[:],
                                     func=mybir.ActivationFunctionType.Relu,
                                     bias=be_t[:])
                h2p = ps1.tile([32, CW], DT.float32, tag="h2")
                nc.tensor.matmul(out=h2p[:], lhsT=wg_t[:], rhs=h1s[:],
                                 start=True, stop=True)
                h2s = pool.tile([32, CW], DT.float32, tag="h2s")
                nc.vector.tensor_copy(out=h2s[:], in_=h2p[:])
                tp = ps3.tile([P, GPB, 32], DT.float32, tag="tp")
                for t in range(GPB):
                    nc.tensor.transpose(out=tp[:, t, :], in_=h2s[:, t * P:(t + 1) * P],
                                        identity=id_t[:])
                blk0 = g * GPB
                nc.vector.tensor_tensor(
                    out=g_loc[:, blk0:blk0 + GPB, :], in0=tp[:],
                    in1=dinvc[:, blk0:blk0 + GPB, None].to_broadcast([P, GPB, 32]),
                    op=mybir.AluOpType.mult)
            # remaining 98 - 96 = 2 tiles
            rem = NGL - (NGL // GPB) * GPB
            if rem:
                c0 = (NGL // GPB) * CW
                h1p = ps1.tile([32, rem * P], DT.float32, tag="h1")
                nc.tensor.matmul(out=h1p[:], lhsT=we_t[:], rhs=xc_sb[:, c0:c0 + rem * P],
                                 start=True, stop=True)
                h1s = pool.tile([32, rem * P], DT.float32, tag="h1s2")
                nc.scalar.activation(out=h1s[:], in_=h1p[:],
                                     func=mybir.ActivationFunctionType.Relu,
                                     bias=be_t[:])
                h2p = ps1.tile([32, rem * P], DT.float32, tag="h2")
                nc.tensor.matmul(out=h2p[:], lhsT=wg_t[:], rhs=h1s[:],
                                 start=True, stop=True)
                h2s = pool.tile([32, rem * P], DT.float32, tag="h2s2")
                nc.vector.tensor_copy(out=h2s[:], in_=h2p[:])
                tp = ps3.tile([P, rem, 32], DT.float32, tag="tp")
                for t in range(rem):
                    nc.tensor.transpose(out=tp[:, t, :], in_=h2s[:, t * P:(t + 1) * P],
                                        identity=id_t[:])
                blk0 = (NGL // GPB) * GPB
                nc.vector.tensor_tensor(
                    out=g_loc[:, blk0:blk0 + rem, :], in0=tp[:],
                    in1=dinvc[:, blk0:blk0 + rem, None].to_broadcast([P, rem, 32]),
                    op=mybir.AluOpType.mult)

            # ---- h = relu(dinv * (s + g_loc) + bgcn); u, v   (in place on s_sb)
            hsum = s_sb[:].rearrange("p (b f) -> p b f", f=32)
            nc.vector.tensor_tensor(out=hsum, in0=hsum, in1=g_loc[:],
                                    op=mybir.AluOpType.add)
            nc.vector.tensor_tensor(
                out=hsum, in0=hsum,
                in1=dinvc[:, :, None].to_broadcast([P, NBLK, 32]),
                op=mybir.AluOpType.mult)
            nc.vector.tensor_tensor(
                out=hsum, in0=hsum,
                in1=bg_t[:].rearrange("p (o f) -> p o f", o=1)
                    .to_broadcast([P, NBLK, 32]),
                op=mybir.AluOpType.add)
            h_t = cons.tile([P, NBLK, 32], DT.float32)
            nc.scalar.activation(out=h_t[:], in_=hsum,
                                 func=mybir.ActivationFunctionType.Relu)
            # u = sum_f h*wu ; v = sum_f h*wv
            for (w_t, o_t) in ((wu_t, u_out), (wv_t, v_out)):
                tmp = pool.tile([P, NBLK, 32], DT.float32, tag="uvtmp")
                nc.vector.tensor_tensor(
                    out=tmp[:], in0=h_t[:],
                    in1=w_t[:].rearrange("p (o f) -> p o f", o=1)
                        .to_broadcast([P, NBLK, 32]),
                    op=mybir.AluOpType.mult)
                red = pool.tile([P, NBLK], DT.float32, tag="uvred")
                nc.vector.tensor_reduce(out=red[:], in_=tmp[:],
                                        axis=mybir.AxisListType.X,
                                        op=mybir.AluOpType.add)
                nc.sync.dma_start(out=o_t[:], in_=red[:])
    _split_multi_waits(nc)
    return nc


def _build_l3():
    """scores = sigmoid(eu + ev + b_edge)."""
    nc = bass.Bass("TRN2", debug=False, num_devices=NC)
    eu = nc.dram_tensor("eu", [P, NJ3], DT.float32, kind="ExternalInput")
    ev = nc.dram_tensor("ev", [P, NJ3], DT.float32, kind="ExternalInput")
    bedge = nc.dram_tensor("bedge", [P, 1], DT.float32, kind="ExternalInput")
    sc = nc.dram_tensor("sc", [P, NJ3], DT.float32, kind="ExternalOutput")
    with tile.TileContext(nc) as tc:
        with tc.tile_pool(name="pool", bufs=1) as pool:
            eu_t = pool.tile([P, NJ3], DT.float32)
            nc.sync.dma_start(out=eu_t[:], in_=eu[:])
            ev_t = pool.tile([P, NJ3], DT.float32)
            nc.sync.dma_start(out=ev_t[:], in_=ev[:])
            b_t = pool.tile([P, 1], DT.float32)
            nc.sync.dma_start(out=b_t[:], in_=bedge[:])
            su = pool.tile([P, NJ3], DT.float32)
            nc.vector.tensor_tensor(out=su[:], in0=eu_t[:], in1=ev_t[:],
                                    op=mybir.AluOpType.add)
            sg = pool.tile([P, NJ3], DT.float32)
            nc.scalar.activation(out=sg[:], in_=su[:],
                                 func=mybir.ActivationFunctionType.Sigmoid,
                                 bias=b_t[:])
            nc.sync.dma_start(out=sc[:], in_=sg[:])
    _split_multi_waits(nc)
    return nc


_CACHE = {}


def _get(name, builder):
    if name not in _CACHE:
        _CACHE[name] = builder()
    return _CACHE[name]


def kernel(x_t, x_t_dt, edge_index, W_enc, b_enc, W_gcn, b_gcn, W_edge, b_edge):
    import ml_dtypes
    bf16 = ml_dtypes.bfloat16
    x_t = np.asarray(x_t, dtype=np.float32)
    W_enc = np.asarray(W_enc, np.float32)
    b_enc = np.asarray(b_enc, np.float32)
    W_gcn = np.asarray(W_gcn, np.float32)
    b_gcn = np.asarray(b_gcn, np.float32)
    W_edge = np.asarray(W_edge, np.float32)
    b_edge = np.asarray(b_edge, np.float32)
    src = np.asarray(edge_index[0], np.int64).astype(np.int32)
    dst = np.asarray(edge_index[1], np.int64).astype(np.int32)
    del LAST_EXEC_NS[:]

    iota = np.tile(np.arange(P, dtype=np.float32).astype(bf16).reshape(1, P), (P, 1))

    # ---- shard edges by dst range, bin by dst block (host-side sharding) ----
    core = dst // NPC
    blk_g = dst // P                    # global block id (core*98 + local blk)
    order = np.argsort(blk_g, kind="stable")
    src_o, dst_o = src[order], dst[order]
    blk_o = blk_g[order]
    counts = np.bincount(blk_o, minlength=NC * NBLK)
    assert counts.max() <= BPAD, f"block overflow {counts.max()} > {BPAD}"
    # slot each edge into its block's padded region
    starts = np.zeros(NC * NBLK, np.int64)
    starts[1:] = np.cumsum(counts)[:-1]
    within = np.arange(E) - starts[blk_o]
    slot_g = blk_o * BPAD + within       # global padded slot (core-major)

    # per-core padded edge arrays
    e_src = np.zeros((NC, E2), np.int32)
    e_lo = np.full((NC, E2), 200.0, np.float32)
    c_o = blk_o // NBLK
    slot_l = slot_g - c_o * E2
    e_src[c_o, slot_l] = src_o
    e_lo[c_o, slot_l] = (dst_o % P).astype(np.float32)

    # chunk-major [p, ch] layouts
    def pch(a):      # [NC, E2] -> [NC, P, NCH] with [c, p, ch] = a[c, ch*128+p]
        return np.ascontiguousarray(a.reshape(NC, NCH, P).transpose(0, 2, 1))

    e_lo_pch = pch(e_lo).astype(bf16)

    # ---- L1: degree histogram ----
    nc1 = _get("l1", _build_l1)
    in_maps = [{"dstlo": e_lo_pch[c], "iota_in": iota} for c in range(NC)]
    res1 = run_bass_kernel_spmd(nc1, in_maps, core_ids=list(range(NC)))
    if res1.exec_time_ns:
        LAST_EXEC_NS.append(res1.exec_time_ns)
    deg_full = np.zeros(NPAD, np.float32)
    for c in range(NC):
        d = res1.results[c]["deg_out"]      # [p, blk]
        deg_full[c * NPC:(c + 1) * NPC] = d.T.reshape(-1)

    # ---- L2 prep: halo-exchange per-edge src features ----
    xpad = np.zeros((NPAD, 7), np.float32)
    xpad[:N] = x_t
    ex = xpad[e_src.reshape(-1)].reshape(NC, E2, 7)
    exT = np.ascontiguousarray(ex.transpose(0, 2, 1))          # [NC, 7, E2]
    edeg = pch(deg_full[e_src.reshape(-1)].reshape(NC, E2).astype(np.float32))
    xcT = np.ascontiguousarray(
        xpad.reshape(NC, NPC, 7).transpose(0, 2, 1))           # [NC, 7, NPC]
    degc = np.ascontiguousarray(
        deg_full.reshape(NC, NBLK, P).transpose(0, 2, 1))      # [NC, p, blk]

    wu = W_edge[:32, 0].astype(np.float32)
    wv = W_edge[32:, 0].astype(np.float32)
    common = {
        "wenc": W_enc, "benc": b_enc.reshape(32, 1),
        "wgcn": W_gcn, "bgcn_r": np.tile(b_gcn.reshape(1, 32), (P, 1)),
        "wu_r": np.tile(wu.reshape(1, 32), (P, 1)),
        "wv_r": np.tile(wv.reshape(1, 32), (P, 1)),
        "iota_in": iota, "id32": np.eye(32, dtype=np.float32),
    }
    nc2 = _get("l2", _build_l2)
    in_maps = [dict(common, exT=exT[c], edeg=edeg[c], dstlo=e_lo_pch[c],
                    xcT=xcT[c], degc=degc[c]) for c in range(NC)]
    res2 = run_bass_kernel_spmd(nc2, in_maps, core_ids=list(range(NC)))
    if res2.exec_time_ns:
        LAST_EXEC_NS.append(res2.exec_time_ns)
    u_full = np.zeros(NPAD, np.float32)
    v_full = np.zeros(NPAD, np.float32)
    for c in range(NC):
        u_full[c * NPC:(c + 1) * NPC] = res2.results[c]["u_out"].T.reshape(-1)
        v_full[c * NPC:(c + 1) * NPC] = res2.results[c]["v_out"].T.reshape(-1)

    # ---- L3: edge scorer ----
    # original edge order; core c takes edges [c*E3, (c+1)*E3)
    eu = u_full[src].reshape(NC, NJ3, P).transpose(0, 2, 1)
    ev = v_full[dst].reshape(NC, NJ3, P).transpose(0, 2, 1)
    eu = np.ascontiguousarray(eu)
    ev = np.ascontiguousarray(ev)
    nc3 = _get("l3", _build_l3)
    bvec = np.full((P, 1), float(b_edge.reshape(-1)[0]), np.float32)
    in_maps = [{"eu": eu[c], "ev": ev[c], "bedge": bvec} for c in range(NC)]
    res3 = run_bass_kernel_spmd(nc3, in_maps, core_ids=list(range(NC)))
    if res3.exec_time_ns:
        LAST_EXEC_NS.append(res3.exec_time_ns)
    scores = np.zeros(E, np.float32)
    for c in range(NC):
        sc = res3.results[c]["sc"]          # [p, j]
        scores[c * E3:(c + 1) * E3] = sc.T.reshape(-1)
    return scores
